# revision 1
# baseline (speedup 1.0000x reference)
"""Trainium2 Bass kernel for nn_DRModel (embedding-bag + GRU + L1-normalized
vocab projection + softmax), 8-core SPMD.

Sharding:
  - Vocab dim V split into 8 contiguous chunks of 6250 (tensor-parallel);
    each core normalizes/transposes its chunk and computes its [B*S, 6250]
    logits/softmax slab.  Softmax denominators are globally all-reduced in
    groups so output DMA can start early.
  - Gather+pooling is data-parallel over batch (8 batches per core), then one
    AllGather replicates pooled sequence to every core.
  - GRU runs replicated (same serial latency everywhere); its per-step output
    feeds the chunked logits phase so the two overlap.
"""
import sys
import numpy as np

sys.path.insert(0, "/opt/trn_rl_repo")

V, D, B, S, K = 50000, 128, 64, 20, 20
NC = 8
VC = V // NC            # 6250 vocab rows per core
BL = B // NC            # 8 batches per core
SLOTS = S * BL * K      # 3200 gather slots per core
NT = SLOTS // 128       # 25 gather tiles of 128 rows
GROUPS = S * BL         # 160 pooled (s, b_local) groups per core
MCH = (B * S) // 128    # 10 M-chunks of 128 output rows (2 GRU steps each)
P = 128
# chunk index groups for the softmax-denominator all-reduces
AR_GROUPS = [[0], [1, 2], [3, 4, 5], [6, 7], [8, 9]]

_CACHE = {}


def _build(no_cc=False, ar_groups=None, use_hp=True, ablate=(), mm_dtype="f32r"):
    import concourse.bass as bass
    import concourse.bacc as bacc
    import concourse.mybir as mybir
    import concourse.tile as tile
    from concourse.masks import make_identity

    fp32 = mybir.dt.float32
    i32 = mybir.dt.int32
    Alu = mybir.AluOpType
    Act = mybir.ActivationFunctionType

    nc = bacc.Bacc("TRN2", target_bir_lowering=False, debug=False,
                   enable_asserts=False, num_devices=NC)

    emb_full = nc.dram_tensor("emb_full", [V, D], fp32, kind="ExternalInput")
    emb_chunk = nc.dram_tensor("emb_chunk", [VC, D], fp32, kind="ExternalInput")
    gidx16 = nc.dram_tensor("gidx16", [P, SLOTS // 16], mybir.dt.int16, kind="ExternalInput")
    gdole = nc.dram_tensor("gdole", [P, NT], fp32, kind="ExternalInput")
    gdolo = nc.dram_tensor("gdolo", [P, NT], fp32, kind="ExternalInput")
    pat = nc.dram_tensor("pat", [P, 5, 32], fp32, kind="ExternalInput")
    w_ihT = nc.dram_tensor("w_ihT", [P, 3 * D], fp32, kind="ExternalInput")
    w_hhT = nc.dram_tensor("w_hhT", [P, 3 * D], fp32, kind="ExternalInput")
    b_rz = nc.dram_tensor("b_rz", [P, 2], fp32, kind="ExternalInput")   # 0.5*(b_ih+b_hh) r|z
    b_hn_row = nc.dram_tensor("b_hn_row", [1, P], fp32, kind="ExternalInput")  # b_hh[2D:3D] as row
    b_in = nc.dram_tensor("b_in", [P, 1], fp32, kind="ExternalInput")   # b_ih[2D:3D]
    h0T = nc.dram_tensor("h0T", [P, B], fp32, kind="ExternalInput")
    out_c = nc.dram_tensor("out_c", [B, S, VC], fp32, kind="ExternalOutput")

    RG = [list(range(NC))]

    with tile.TileContext(nc) as tc:
        import contextlib
        groups = ar_groups if ar_groups is not None else AR_GROUPS
        with contextlib.ExitStack() as ctx:
            cpool = ctx.enter_context(tc.tile_pool(name="consts", bufs=1))
            npool = ctx.enter_context(tc.tile_pool(name="normp", bufs=3))
            gpool = ctx.enter_context(tc.tile_pool(name="gath", bufs=4))
            spool = ctx.enter_context(tc.tile_pool(name="small", bufs=2))
            hpool = ctx.enter_context(tc.tile_pool(name="gru", bufs=2))
            expool = ctx.enter_context(tc.tile_pool(name="expb", bufs=5))
            dram = ctx.enter_context(tc.tile_pool(name="dram", bufs=1, space="DRAM"))
            ps_g = ctx.enter_context(tc.tile_pool(name="ps_g", bufs=2, space="PSUM"))
            ps_m = ctx.enter_context(tc.tile_pool(name="ps_m", bufs=2, space="PSUM"))

            # ---- constants / small inputs -------------------------------
            ident = cpool.tile([P, P], fp32)
            make_identity(nc, ident[:])
            gidx_t = cpool.tile([P, SLOTS // 16], mybir.dt.int16)
            nc.sync.dma_start(out=gidx_t[:], in_=gidx16[:])
            gdole_t = cpool.tile([P, NT], fp32)
            nc.sync.dma_start(out=gdole_t[:], in_=gdole[:])
            gdolo_t = cpool.tile([P, NT], fp32)
            nc.sync.dma_start(out=gdolo_t[:], in_=gdolo[:])
            ones1 = cpool.tile([1, B], fp32)
            nc.gpsimd.memset(ones1[:], 1.0)
            pat_t = cpool.tile([P, 5, 32], fp32)
            nc.sync.dma_start(out=pat_t[:], in_=pat[:])
            wih_t = cpool.tile([P, 3 * D], fp32)
            nc.sync.dma_start(out=wih_t[:], in_=w_ihT[:])
            whh_t = cpool.tile([P, 3 * D], fp32)
            nc.sync.dma_start(out=whh_t[:], in_=w_hhT[:])
            brz_t = cpool.tile([P, 2], fp32)
            nc.sync.dma_start(out=brz_t[:], in_=b_rz[:])
            bhn_row_t = cpool.tile([1, P], fp32)
            nc.sync.dma_start(out=bhn_row_t[:], in_=b_hn_row[:])
            bin_t = cpool.tile([P, 1], fp32)
            nc.sync.dma_start(out=bin_t[:], in_=b_in[:])
            h0_t = cpool.tile([P, B], fp32)
            nc.sync.dma_start(out=h0_t[:], in_=h0T[:])

            mmdt = mybir.dt.float32r if mm_dtype == "f32r" else fp32
            itemT = cpool.tile([P, VC], mmdt)       # normalized emb chunk, transposed
            pooledT = cpool.tile([P, S * B], fp32)  # pooled basket seq, transposed
            userT = [cpool.tile([P, P], mmdt, name=f"userT{m}") for m in range(MCH)]

            # ---- gather + pool (this core's 8 batches) ------------------
            # pair-index dma_gather: fetch 1KB (2 emb rows) per index; the
            # wanted half is selected by zeroing the other half's weight in
            # the pooling pattern (dollars_even / dollars_odd).
            NBLK_A, NBLK_B = 13, NT - 13
            NIDX_A, NIDX_B = NBLK_A * P, NBLK_B * P
            emb_pairs = emb_full[:].rearrange("(a two) d -> a (two d)", two=2)
            bufA = expool.tile([P, NBLK_A, 2 * D], fp32, tag="exp")
            if "nogather" in ablate:
                nc.gpsimd.memset(bufA[:], 0.5)
            else:
                nc.gpsimd.dma_gather(
                    out_ap=bufA[:], in_ap=emb_pairs,
                    idxs_ap=gidx_t[:, :NIDX_A // 16],
                    num_idxs=NIDX_A, num_idxs_reg=NIDX_A, elem_size=2 * D,
                    single_packet=False)
            bufB = expool.tile([P, NBLK_B, 2 * D], fp32, tag="exp")
            if "nogather" in ablate:
                nc.gpsimd.memset(bufB[:], 0.5)
            else:
                nc.gpsimd.dma_gather(
                    out_ap=bufB[:], in_ap=emb_pairs,
                    idxs_ap=gidx_t[:, NIDX_A // 16:],
                    num_idxs=NIDX_B, num_idxs_reg=NIDX_B, elem_size=2 * D,
                    single_packet=False)
            pool_ps = ps_g.tile([P, GROUPS], fp32, tag="g1")
            for t in range(NT):
                gt = bufA[:, t, :] if t < NBLK_A else bufB[:, t - NBLK_A, :]
                w0 = 32 * (t // 5)
                pat_e = gpool.tile([P, 32], fp32, tag="pe")
                nc.gpsimd.tensor_scalar_mul(out=pat_e[:], in0=pat_t[:, t % 5, :],
                                            scalar1=gdole_t[:, t:t + 1])
                pat_o = gpool.tile([P, 32], fp32, tag="po")
                nc.gpsimd.tensor_scalar_mul(out=pat_o[:], in0=pat_t[:, t % 5, :],
                                            scalar1=gdolo_t[:, t:t + 1])
                nc.tensor.matmul(pool_ps[:, w0:w0 + 32], lhsT=gt[0:P, 0:D],
                                 rhs=pat_e[:], start=(t % 5 == 0), stop=False)
                nc.tensor.matmul(pool_ps[:, w0:w0 + 32], lhsT=gt[0:P, D:2 * D],
                                 rhs=pat_o[:], start=False, stop=(t % 5 == 4))
            pool_part = spool.tile([P, GROUPS], fp32)
            nc.scalar.copy(out=pool_part[:], in_=pool_ps[:])

            agin = dram.tile([P, GROUPS], fp32)
            agout = dram.tile([NC, P, GROUPS], fp32, addr_space="Shared")
            nc.sync.dma_start(out=agin[:], in_=pool_part[:])
            if not no_cc:
                nc.gpsimd.collective_compute(
                    "AllGather", Alu.bypass, replica_groups=RG,
                    ins=[agin[:].opt()], outs=[agout[:].opt()],
                )
            pooled_v = pooledT[:].rearrange("p (s b) -> p s b", s=S)
            for cc in range(NC):
                nc.sync.dma_start(
                    out=pooled_v[:, :, cc * BL:(cc + 1) * BL],
                    in_=(agin[:] if no_cc else agout[cc]).rearrange("p (s b) -> p s b", s=S),
                )


            if "nonorm" in ablate:
                nc.gpsimd.memset(itemT[:], 0.001)
            else:
                # ---- normalize + transpose vocab chunk (emitted last: fills
                # engine idle slots; itemT dependency gates the chunk matmuls) ---
                SUB = 8
                n_sup = VC // (SUB * P)          # 6 super-tiles of 1024 rows
                for u in range(n_sup):
                    r0 = SUB * P * u
                    et = npool.tile([P, SUB, D], fp32, tag="et")
                    nc.sync.dma_start(
                        out=et[:],
                        in_=emb_chunk[r0:r0 + SUB * P, :].rearrange(
                            "(a p) d -> p a d", p=P))
                    l1 = npool.tile([P, SUB], fp32, tag="l1")
                    nc.vector.tensor_reduce(out=l1[:], in_=et[:],
                                            axis=mybir.AxisListType.X, op=Alu.add,
                                            apply_absolute_value=True)
                    inv = npool.tile([P, SUB], fp32, tag="inv")
                    nc.vector.reciprocal(out=inv[:], in_=l1[:])
                    for a in range(SUB):
                        t = SUB * u + a
                        nrm = npool.tile([P, D], fp32, tag="nrm")
                        eng = nc.gpsimd if a % 2 == 0 else nc.vector
                        eng.tensor_scalar_mul(out=nrm[:], in0=et[:, a, :],
                                              scalar1=inv[:, a:a + 1])
                        pt = ps_m.tile([P, 1024], fp32, tag="big")
                        nc.tensor.transpose(out=pt[:, :P], in_=nrm[:], identity=ident[:])
                        dst = itemT[:, r0 + a * P:r0 + (a + 1) * P]
                        if t % 2 == 0:
                            nc.scalar.copy(out=dst, in_=pt[:, :P])
                        else:
                            nc.vector.tensor_copy(out=dst, in_=pt[:, :P])
                # ragged tail: rows 6144..6250 (106 rows)
                r0 = n_sup * SUB * P
                rn = VC - r0
                et2 = npool.tile([P, D], fp32, tag="et2")
                nc.sync.dma_start(out=et2[:rn], in_=emb_chunk[r0:r0 + rn, :])
                l12 = npool.tile([P, 1], fp32, tag="l12")
                nc.vector.tensor_reduce(out=l12[:rn], in_=et2[:rn],
                                        axis=mybir.AxisListType.X, op=Alu.add,
                                        apply_absolute_value=True)
                inv2 = npool.tile([P, 1], fp32, tag="inv2")
                nc.vector.reciprocal(out=inv2[:rn], in_=l12[:rn])
                nrm2 = npool.tile([P, D], fp32, tag="nrm2")
                nc.gpsimd.tensor_scalar_mul(out=nrm2[:rn], in0=et2[:rn], scalar1=inv2[:rn])
                pt2 = ps_m.tile([P, 1024], fp32, tag="big")
                nc.tensor.transpose(out=pt2[:, :rn], in_=nrm2[:rn], identity=ident[:rn, :rn])
                nc.scalar.copy(out=itemT[:, r0:r0 + rn], in_=pt2[:, :rn])


            # ---- GRU + chunked logits/softmax ---------------------------
            all_sums = spool.tile([P, MCH], fp32, bufs=1)
            inv_sums = spool.tile([P, MCH], fp32, bufs=1)
            regions = []
            n0 = 0
            while n0 < VC:
                nn = min(1024, VC - n0)
                regions.append((n0, nn))
                n0 += nn

            prev = h0_t[:, :].bitcast(fp32)
            ar_group_of_chunk = {}
            for gi, grp in enumerate(groups):
                for m in grp:
                    ar_group_of_chunk[m] = gi

            def _mmcast(ap):
                return ap

            def emit_chunk(m):
                part_sums = spool.tile([P, len(regions)], fp32, tag="psums")
                exp_m = expool.tile([P, VC], fp32, tag="exp", name=f"exp{m}")
                if "nomm" in ablate:
                    (nc.vector if m % 2 else nc.gpsimd).memset(exp_m[:], 0.1)
                    return exp_m
                for j, (r0, rn) in enumerate(regions):
                    pb = ps_m.tile([P, 1024], fp32, tag="big")
                    n1 = min(512, rn)
                    nc.tensor.matmul(pb[:, 0:n1], lhsT=_mmcast(userT[m][:]),
                                     rhs=_mmcast(itemT[:, r0:r0 + n1]),
                                     start=True, stop=True)
                    if rn > 512:
                        nc.tensor.matmul(pb[:, 512:rn], lhsT=_mmcast(userT[m][:]),
                                         rhs=_mmcast(itemT[:, r0 + 512:r0 + rn]),
                                         start=True, stop=True)
                    nc.scalar.activation(out=exp_m[:, r0:r0 + rn], in_=pb[:, :rn],
                                         func=Act.Exp,
                                         accum_out=part_sums[:, j:j + 1])
                nc.vector.tensor_reduce(out=all_sums[:, m:m + 1], in_=part_sums[:],
                                        axis=mybir.AxisListType.X, op=Alu.add)
                return exp_m

            def emit_ar(gi):
                if "nomm" in ablate:
                    return
                grp = groups[gi]
                ng = len(grp)
                g0 = grp[0]
                arin = dram.tile([P, ng], fp32, name=f"arin{gi}")
                arout = dram.tile([P, ng], fp32, addr_space="Shared", name=f"arout{gi}")
                nc.sync.dma_start(out=arin[:], in_=all_sums[:, g0:g0 + ng])
                if not no_cc:
                    nc.gpsimd.collective_compute(
                        "AllReduce", Alu.add, replica_groups=RG,
                        ins=[arin[:].opt()], outs=[arout[:].opt()],
                    )
                gs = spool.tile([P, ng], fp32, tag="gs", name=f"gs{gi}")
                nc.sync.dma_start(out=gs[:], in_=(arin[:] if no_cc else arout[:]))
                nc.vector.reciprocal(out=inv_sums[:, g0:g0 + ng], in_=gs[:])

            def emit_scale_out(m, exp_m):
                if "nooutdma" in ablate:
                    return
                if "nomm" in ablate:
                    ov = out_c[:, 2 * m:2 * m + 2, :].rearrange("b s v -> s b v")
                    nc.sync.dma_start(out=ov, in_=exp_m[:])
                    return
                if m % 3 == 2:
                    nc.gpsimd.tensor_scalar_mul(out=exp_m[:], in0=exp_m[:],
                                                scalar1=inv_sums[:, m:m + 1])
                else:
                    nc.vector.tensor_scalar_mul(out=exp_m[:], in0=exp_m[:],
                                                scalar1=inv_sums[:, m:m + 1])
                ov = out_c[:, 2 * m:2 * m + 2, :].rearrange("b s v -> s b v")
                nc.sync.dma_start(out=ov, in_=exp_m[:])

            exp_tiles = {}
            done_groups = set()
            if "nogru" in ablate:
                for m in range(MCH):
                    nc.gpsimd.memset(userT[m][:], 0.01)
                for m in range(MCH):
                    exp_tiles_p = emit_chunk(m)
                    gi = ar_group_of_chunk[m]
                    if m == groups[gi][-1]:
                        emit_ar(gi)
                        for mm_ in groups[gi]:
                            pass
                        emit_scale_out(m, exp_tiles_p) if False else None
                    emit_scale_out(m, exp_tiles_p)
            for t in range(S):
                if "nogru" in ablate:
                    break
                m, half = divmod(t, 2)
                hp = tc.high_priority() if use_hp else None
                if hp: hp.__enter__()
                x_t = pooledT[:, t * B:(t + 1) * B]
                prz = ps_g.tile([P, 2 * B], fp32, tag="g1", name=f"prz{t}")
                nc.tensor.matmul(prz[:, 0:B], lhsT=wih_t[:, 0:D], rhs=x_t,
                                 start=True, stop=False)
                nc.tensor.matmul(prz[:, 0:B], lhsT=whh_t[:, 0:D],
                                 rhs=prev.bitcast(fp32),
                                 start=False, stop=True)
                nc.tensor.matmul(prz[:, B:2 * B], lhsT=wih_t[:, D:2 * D], rhs=x_t,
                                 start=True, stop=False)
                nc.tensor.matmul(prz[:, B:2 * B], lhsT=whh_t[:, D:2 * D],
                                 rhs=prev.bitcast(fp32),
                                 start=False, stop=True)
                pn = ps_g.tile([P, 2 * B], fp32, tag="g2", name=f"pn{t}")
                nc.tensor.matmul(pn[:, 0:B], lhsT=wih_t[:, 2 * D:3 * D], rhs=x_t,
                                 start=True, stop=True)
                nc.tensor.matmul(pn[:, B:2 * B], lhsT=whh_t[:, 2 * D:3 * D],
                                 rhs=prev.bitcast(fp32),
                                 start=True, stop=False)
                nc.tensor.matmul(pn[:, B:2 * B], lhsT=bhn_row_t[:], rhs=ones1[:],
                                 start=False, stop=True)
                # r = sigmoid(i_r + h_r + b) via 0.5*tanh(0.5*x + 0.5*b) + 0.5
                rt = hpool.tile([P, B], fp32, tag="rt")
                nc.scalar.activation(out=rt[:], in_=prz[:, 0:B], func=Act.Tanh,
                                     bias=brz_t[:, 0:1], scale=0.5)
                nc.vector.tensor_scalar(out=rt[:], in0=rt[:], scalar1=0.5,
                                        scalar2=0.5, op0=Alu.mult, op1=Alu.add)
                zt = hpool.tile([P, B], fp32, tag="zt")
                nc.scalar.activation(out=zt[:], in_=prz[:, B:2 * B], func=Act.Tanh,
                                     bias=brz_t[:, 1:2], scale=0.5)
                nc.vector.tensor_scalar(out=zt[:], in0=zt[:], scalar1=0.5,
                                        scalar2=0.5, op0=Alu.mult, op1=Alu.add)
                t1 = hpool.tile([P, B], fp32, tag="t1")
                nc.vector.tensor_tensor(out=t1[:], in0=rt[:], in1=pn[:, B:2 * B], op=Alu.mult)
                t2 = hpool.tile([P, B], fp32, tag="t2")
                nc.vector.tensor_tensor(out=t2[:], in0=t1[:], in1=pn[:, 0:B], op=Alu.add)
                nt_ = hpool.tile([P, B], fp32, tag="nt")
                nc.scalar.activation(out=nt_[:], in_=t2[:], func=Act.Tanh,
                                     bias=bin_t[:, 0:1])
                dd = hpool.tile([P, B], fp32, tag="dd")
                nc.vector.tensor_tensor(out=dd[:], in0=prev.bitcast(fp32),
                                        in1=nt_[:], op=Alu.subtract)
                ee = hpool.tile([P, B], fp32, tag="ee")
                nc.vector.tensor_tensor(out=ee[:], in0=zt[:], in1=dd[:], op=Alu.mult)
                hdst = userT[m][:, half * B:(half + 1) * B]
                nc.vector.tensor_tensor(out=hdst, in0=nt_[:], in1=ee[:], op=Alu.add)
                prev = hdst
                if hp: hp.__exit__(None, None, None)

                if half == 1:
                    exp_tiles[m] = emit_chunk(m)
                    gi = ar_group_of_chunk[m]
                    if m == groups[gi][-1]:
                        emit_ar(gi)
                        for mm_ in groups[gi]:
                            emit_scale_out(mm_, exp_tiles.pop(mm_))
                        done_groups.add(gi)

    nc.compile()
    return nc


def _prep_inputs(basket_items, basket_dollars, hidden, emb, W_ih, W_hh, b_ih, b_hh):
    emb = np.ascontiguousarray(np.asarray(emb, dtype=np.float32))
    items = np.asarray(basket_items).astype(np.int32)
    dollars = np.asarray(basket_dollars, dtype=np.float32)
    W_ihT = np.ascontiguousarray(np.asarray(W_ih, dtype=np.float32).T)  # [128, 384]
    W_hhT = np.ascontiguousarray(np.asarray(W_hh, dtype=np.float32).T)
    b_ih = np.asarray(b_ih, dtype=np.float32)
    b_hh = np.asarray(b_hh, dtype=np.float32)
    b_rz = 0.5 * (b_ih[:2 * D] + b_hh[:2 * D])
    b_rz = np.ascontiguousarray(b_rz.reshape(2, D).T)                    # [128, 2]
    b_hn_row = np.ascontiguousarray(b_hh[2 * D:].reshape(1, D))
    b_in = np.ascontiguousarray(b_ih[2 * D:].reshape(D, 1))
    h0T = np.ascontiguousarray(np.asarray(hidden, dtype=np.float32)[0].T)  # [128, 64]

    # pooling pattern, periodic with lcm(128, 20) = 640 slots = 5 tiles:
    # tile t uses pat[:, t % 5, :] into psum window 32 * (t // 5).
    j = np.arange(5 * P)
    pat = np.zeros((P, 5, 32), dtype=np.float32)
    pat[j % P, j // P, j // K] = 1.0

    common = dict(emb_full=emb, pat=pat, w_ihT=W_ihT, w_hhT=W_hhT,
                  b_rz=b_rz, b_hn_row=b_hn_row, b_in=b_in, h0T=h0T)
    in_maps = []
    for c in range(NC):
        items_c = items[c * BL:(c + 1) * BL]          # [8, S, K]
        dol_c = dollars[c * BL:(c + 1) * BL]
        idx_flat = items_c.transpose(1, 0, 2).reshape(-1)   # s-major slots
        dol_flat = dol_c.transpose(1, 0, 2).reshape(-1) * (1.0 / K)
        parity = (idx_flat & 1).astype(np.float32)
        pair_idx = (idx_flat >> 1).astype(np.int16)
        # dma_gather index layout: [16, n/16] with flat[c*16+p] at [p, c],
        # replicated across the 8 Q7 cores (rows 16..127).
        wrapped = pair_idx.reshape(SLOTS // 16, 16).T        # [16, n/16]
        gidx16 = np.ascontiguousarray(np.tile(wrapped, (8, 1)))
        gdole = np.ascontiguousarray(
            (dol_flat * (1.0 - parity)).reshape(NT, P).T.astype(np.float32))
        gdolo = np.ascontiguousarray(
            (dol_flat * parity).reshape(NT, P).T.astype(np.float32))
        emb_chunk = np.ascontiguousarray(emb[c * VC:(c + 1) * VC])
        in_maps.append(dict(common, emb_chunk=emb_chunk, gidx16=gidx16,
                            gdole=gdole, gdolo=gdolo))
    return in_maps


def kernel(basket_items, basket_dollars, hidden, emb, W_ih, W_hh, b_ih, b_hh,
           _want_trace=False):
    from concourse.bass_utils import run_bass_kernel_spmd

    if "nc" not in _CACHE:
        _CACHE["nc"] = _build()
    nc = _CACHE["nc"]

    in_maps = _prep_inputs(basket_items, basket_dollars, hidden, emb,
                           W_ih, W_hh, b_ih, b_hh)
    res = run_bass_kernel_spmd(nc, in_maps, core_ids=list(range(NC)),
                               trace=_want_trace)
    _CACHE["last_result"] = res
    out = np.concatenate([r["out_c"] for r in res.results], axis=2)
    return out



# revision 11
# speedup vs baseline: 2.2849x; 2.2849x over previous
"""Trainium2 Bass kernel for nn_DRModel (embedding-bag + GRU + L1-normalized
vocab projection + softmax), 8-core SPMD.

Sharding:
  - Vocab dim V split into 8 chunks of 6250 (tensor-parallel); each core
    normalizes/transposes its chunk (bf16) and computes its [B*S, 6250]
    logits/exp slab.  Softmax denominators are exchanged with small
    AllGathers (grouped) + a local reduce, so output DMA starts early.
  - Gather+pooling is data-parallel over batch (8 batches per core); one
    AllGather replicates the pooled sequence to every core.
  - GRU runs replicated; its per-step output feeds the chunked logits
    phase so the two overlap.

All per-core side inputs are packed into ONE [128, MC] f32 tensor ("misc"):
per-exec dispatch overhead scales strongly with the number of NEFF input
tensors (~30 us each through the PJRT relay), so the kernel takes exactly
two inputs (emb + misc) and returns one bf16 output (converted to f32 on
the host after the gather).
"""
import sys
import numpy as np

sys.path.insert(0, "/opt/trn_rl_repo")

V, D, B, S, K = 50000, 128, 64, 20, 20
NC = 8
VC = V // NC            # 6250 vocab rows per core
BL = B // NC            # 8 batches per core
SLOTS = S * BL * K      # 3200 gather slots per core
NT = SLOTS // 128       # 25 gather tiles of 128 rows
GROUPS = S * BL         # 160 pooled (s, b_local) groups per core
MCH = (B * S) // 128    # 10 M-chunks of 128 output rows (2 GRU steps each)
P = 128
# chunk index groups for the softmax-denominator sum-AllGathers
AR_GROUPS = [[0, 1, 2, 3], [4, 5, 6], [7, 8, 9]]
# chunk regions: 4 x 1536 (3 psum banks each) + 106 tail
REGIONS = [(0, 1536), (1536, 1536), (3072, 1536), (4608, 1536), (6144, 106)]

# misc tensor column layout (f32 cols; [128, MC])
PAT_E0 = 0            # [128, 25, 32]
PAT_O0 = 800
WIH0 = 1600           # [128, 384]
WHH0 = 1984
H00 = 2368            # [128, 64]
BIN0 = 2432           # [128, 1]
GIDX0 = 2433          # int16 [128, 200] packed in 100 f32 cols
BRZ0 = 2533           # rows 0:2, [2, 128]
SEL0 = 2661           # rows 0:2, [2, 128]
BHN0 = 2789           # row 0, [1, 128]
# per-core emb chunk, pre-arranged in et layout: block u holds
# misc[p, CH0+u*1024+a*128+d] = emb[c*VC + u*1024 + a*128 + p, d]
CH0 = 2920            # 6 supertiles x 1024 cols
CT0 = CH0 + 6144      # tail: misc[p, CT0+d] = emb[c*VC + 6144 + p, d], p<106
MC = CT0 + 128

_CACHE = {}


def _build(no_cc=False, ar_groups=None, ablate=()):
    import concourse.bass as bass
    import concourse.bacc as bacc
    import concourse.mybir as mybir
    import concourse.tile as tile

    fp32 = mybir.dt.float32
    bf16 = mybir.dt.bfloat16
    i16 = mybir.dt.int16
    Alu = mybir.AluOpType
    Act = mybir.ActivationFunctionType

    nc = bacc.Bacc("TRN2", target_bir_lowering=False, debug=False,
                   enable_asserts=False, num_devices=NC)

    emb_full = nc.dram_tensor("emb_full", [V, D], fp32, kind="ExternalInput")
    misc = nc.dram_tensor("misc", [P, MC], fp32, kind="ExternalInput")
    out_c = nc.dram_tensor("out_c", [B, S, VC], bf16, kind="ExternalOutput")

    RG = [list(range(NC))]
    groups = ar_groups if ar_groups is not None else AR_GROUPS

    with tile.TileContext(nc) as tc:
        import contextlib
        with contextlib.ExitStack() as ctx:
            cpool = ctx.enter_context(tc.tile_pool(name="consts", bufs=1))
            npool = ctx.enter_context(tc.tile_pool(name="normp", bufs=3))
            spool = ctx.enter_context(tc.tile_pool(name="small", bufs=2))
            hpool = ctx.enter_context(tc.tile_pool(name="gru", bufs=2))
            expool = ctx.enter_context(tc.tile_pool(name="expb", bufs=7))
            dram = ctx.enter_context(tc.tile_pool(name="dram", bufs=1, space="DRAM"))
            ps_g = ctx.enter_context(tc.tile_pool(name="ps_g", bufs=2, space="PSUM"))
            ps_m = ctx.enter_context(tc.tile_pool(name="ps_m", bufs=2, space="PSUM"))

            # ---- packed consts ------------------------------------------
            misc_t = cpool.tile([P, MC], fp32)
            nc.scalar.dma_start(out=misc_t[:], in_=misc[:])
            pat_e = misc_t[:, PAT_E0:PAT_E0 + 800].rearrange(
                "p (t w) -> p t w", w=32)
            pat_o = misc_t[:, PAT_O0:PAT_O0 + 800].rearrange(
                "p (t w) -> p t w", w=32)
            wih = misc_t[:, WIH0:WIH0 + 3 * D]
            whh = misc_t[:, WHH0:WHH0 + 3 * D]
            h0v = misc_t[:, H00:H00 + B]
            binv = misc_t[:, BIN0:BIN0 + 1]
            gidx = misc_t[:, GIDX0:GIDX0 + 100].bitcast(i16)   # [P, 200]
            brz2 = misc_t[0:2, BRZ0:BRZ0 + P]
            sel2 = misc_t[0:2, SEL0:SEL0 + P]
            bhn = misc_t[0:1, BHN0:BHN0 + P]

            ones1 = cpool.tile([1, B], fp32)
            nc.gpsimd.memset(ones1[:], 1.0)
            zbias = cpool.tile([P, 1], fp32)
            nc.gpsimd.memset(zbias[:], 0.0)

            itemT = cpool.tile([P, VC + 22], bf16)   # normalized emb chunk^T
            pooled = cpool.tile([P, NC, S, BL], fp32)
            userT = [cpool.tile([P, P], fp32, name=f"userT{m}") for m in range(MCH)]

            # ---- gather + pool (this core's 8 batches) ------------------
            # pair-index dma_gather: fetch 1KB (2 emb rows) per index; the
            # wanted half is selected by zeroing the other half's weight in
            # the host-prescaled pooling patterns (pat_e / pat_o).
            NBLK_A, NBLK_B = 13, NT - 13
            NIDX_A, NIDX_B = NBLK_A * P, NBLK_B * P
            emb_pairs = emb_full[:].rearrange("(a two) d -> a (two d)", two=2)
            bufA = expool.tile([P, NBLK_A, 2 * D], fp32, tag="exp")
            if "nogather" in ablate:
                nc.gpsimd.memset(bufA[:], 0.5)
            else:
                nc.gpsimd.dma_gather(
                    out_ap=bufA[:], in_ap=emb_pairs,
                    idxs_ap=gidx[:, :NIDX_A // 16],
                    num_idxs=NIDX_A, num_idxs_reg=NIDX_A, elem_size=2 * D,
                    single_packet=False)
            bufB = expool.tile([P, NBLK_B, 2 * D], fp32, tag="exp")
            if "nogather" in ablate:
                nc.gpsimd.memset(bufB[:], 0.5)
            else:
                nc.gpsimd.dma_gather(
                    out_ap=bufB[:], in_ap=emb_pairs,
                    idxs_ap=gidx[:, NIDX_A // 16:],
                    num_idxs=NIDX_B, num_idxs_reg=NIDX_B, elem_size=2 * D,
                    single_packet=False)
            pool_ps = ps_g.tile([P, GROUPS], fp32, tag="g1")
            for t in range(NT):
                gt = bufA[:, t, :] if t < NBLK_A else bufB[:, t - NBLK_A, :]
                w0 = 32 * (t // 5)
                nc.tensor.matmul(pool_ps[:, w0:w0 + 32], lhsT=gt[0:P, 0:D],
                                 rhs=pat_e[:, t, :], start=(t % 5 == 0), stop=False)
                nc.tensor.matmul(pool_ps[:, w0:w0 + 32], lhsT=gt[0:P, D:2 * D],
                                 rhs=pat_o[:, t, :], start=False, stop=(t % 5 == 4))
            pool_part = spool.tile([P, GROUPS], fp32)
            nc.vector.tensor_copy(out=pool_part[:], in_=pool_ps[:])

            agin = dram.tile([P, GROUPS], fp32)
            agout = dram.tile([NC, P, GROUPS], fp32, addr_space="Shared")
            nc.sync.dma_start(out=agin[:], in_=pool_part[:])
            if not no_cc:
                nc.gpsimd.collective_compute(
                    "AllGather", Alu.bypass, replica_groups=RG,
                    ins=[agin[:].opt()], outs=[agout[:].opt()],
                )
            # one DMA: [NC, P, (s b)] -> [P, NC, s, b]; per-(p,c) 640B runs
            if no_cc:
                for cc in range(NC):
                    nc.sync.dma_start(
                        out=pooled[:, cc],
                        in_=agin[:].rearrange("p (s b) -> p s b", s=S))
            else:
                nc.sync.dma_start(
                    out=pooled[:],
                    in_=agout[:].rearrange("c p (s b) -> p c s b", s=S))

            # ---- normalize + transpose vocab chunk to bf16 itemT --------
            if "nonorm" in ablate:
                nc.gpsimd.memset(itemT[:], 0.001)
            else:
                SUB = 8
                n_sup = VC // (SUB * P)          # 6 super-tiles of 1024 rows
                for u in range(n_sup):
                    r0 = SUB * P * u
                    # et layout comes packed inside misc (no extra DMA):
                    # et[p, a, d] = emb[c*VC + r0 + a*128 + p, d]
                    et = misc_t[:, CH0 + r0:CH0 + r0 + SUB * D].rearrange(
                        "p (a d) -> p a d", d=D)
                    l1 = npool.tile([P, SUB], fp32, tag="l1")
                    nc.vector.tensor_reduce(out=l1[:], in_=et[:],
                                            axis=mybir.AxisListType.X, op=Alu.add,
                                            apply_absolute_value=True)
                    inv = npool.tile([P, SUB], fp32, tag="inv")
                    nc.vector.reciprocal(out=inv[:], in_=l1[:])
                    nrm = npool.tile([P, SUB * D], bf16, tag="nrm")
                    for a in range(SUB):
                        eng = nc.gpsimd if a % 2 == 0 else nc.vector
                        eng.tensor_scalar_mul(out=nrm[:, a * D:(a + 1) * D],
                                              in0=et[:, a, :],
                                              scalar1=inv[:, a:a + 1])
                    nc.sync.dma_start_transpose(
                        out=itemT[:, r0:r0 + SUB * P].rearrange(
                            "p (a j) -> p a j", j=P),
                        in_=nrm[:])
                # ragged tail: rows 6144..6250 (106 rows)
                r0 = n_sup * SUB * P
                rn = VC - r0
                et2 = misc_t[:, CT0:CT0 + D]    # rows >= rn are zeros
                l12 = npool.tile([P, 1], fp32, tag="l12")
                nc.vector.tensor_reduce(out=l12[:rn], in_=et2[:rn],
                                        axis=mybir.AxisListType.X, op=Alu.add,
                                        apply_absolute_value=True)
                inv2 = npool.tile([P, 1], fp32, tag="inv2")
                nc.vector.reciprocal(out=inv2[:rn], in_=l12[:rn])
                nrm2 = npool.tile([P, D], bf16, tag="nrm2")
                nc.gpsimd.memset(nrm2[:], 0.0)
                nc.vector.tensor_scalar_mul(out=nrm2[:rn], in0=et2[:rn],
                                            scalar1=inv2[:rn])
                nc.sync.dma_start_transpose(
                    out=itemT[:, r0:r0 + P].rearrange("p (a j) -> p a j", j=P),
                    in_=nrm2[:])

            # ---- GRU + chunked logits/softmax ---------------------------
            all_sums = spool.tile([P, MCH], fp32, bufs=1)
            inv_sums = spool.tile([P, MCH], fp32, bufs=1)

            prev = h0v
            gi_of_chunk = {}
            for gi_, grp in enumerate(groups):
                for m in grp:
                    gi_of_chunk[m] = gi_

            def emit_chunk(m, ubf):
                part_sums = spool.tile([P, len(REGIONS)], fp32, tag="psums")
                exp_m = expool.tile([P, VC], bf16, tag="exp", name=f"exp{m}")
                if "nomm" in ablate:
                    nc.gpsimd.memset(exp_m[:], 0.1)
                    nc.vector.tensor_reduce(
                        out=all_sums[:, m:m + 1], in_=exp_m[:, 0:8],
                        axis=mybir.AxisListType.X, op=Alu.add)
                    return exp_m
                for j, (r0, rn) in enumerate(REGIONS):
                    pb = ps_m.tile([P, 1536], fp32, tag="big")
                    c0 = 0
                    while c0 < rn:
                        cw = min(512, rn - c0)
                        nc.tensor.matmul(pb[:, c0:c0 + cw], lhsT=ubf[:],
                                         rhs=itemT[:, r0 + c0:r0 + c0 + cw],
                                         start=True, stop=True)
                        c0 += cw
                    nc.scalar.activation(out=exp_m[:, r0:r0 + rn], in_=pb[:, :rn],
                                         func=Act.Exp,
                                         accum_out=part_sums[:, j:j + 1])
                nc.vector.tensor_reduce(out=all_sums[:, m:m + 1], in_=part_sums[:],
                                        axis=mybir.AxisListType.X, op=Alu.add)
                return exp_m

            def emit_ar(gi_):
                grp = groups[gi_]
                ng = len(grp)
                g0 = grp[0]
                arin = dram.tile([P, ng], fp32, name=f"arin{gi_}")
                arout = dram.tile([NC, P, ng], fp32, addr_space="Shared",
                                  name=f"arout{gi_}")
                nc.gpsimd.dma_start(out=arin[:], in_=all_sums[:, g0:g0 + ng])
                if not no_cc:
                    nc.gpsimd.collective_compute(
                        "AllGather", Alu.bypass, replica_groups=RG,
                        ins=[arin[:].opt()], outs=[arout[:].opt()],
                    )
                gsg = spool.tile([P, NC, ng], fp32, tag="gs", name=f"gs{gi_}")
                if no_cc:
                    for cc in range(NC):
                        nc.sync.dma_start(out=gsg[:, cc], in_=arin[:])
                else:
                    nc.sync.dma_start(out=gsg[:],
                                      in_=arout[:].rearrange("c p g -> p c g"))
                gsum = spool.tile([P, ng], fp32, tag="gsum", name=f"gsum{gi_}")
                nc.vector.tensor_reduce(
                    out=gsum[:], in_=gsg[:].rearrange("p c g -> p g c"),
                    axis=mybir.AxisListType.X, op=Alu.add)
                nc.vector.reciprocal(out=inv_sums[:, g0:g0 + ng], in_=gsum[:])

            def emit_scale_out(m, exp_m):
                if "nooutdma" in ablate:
                    return
                nc.vector.tensor_scalar_mul(out=exp_m[:], in0=exp_m[:],
                                            scalar1=inv_sums[:, m:m + 1])
                ov = out_c[:, 2 * m:2 * m + 2, :].rearrange("b s v -> s b v")
                nc.sync.dma_start(out=ov, in_=exp_m[:])

            exp_tiles = {}
            for t in range(S):
                m, half = divmod(t, 2)
                x_t = pooled[:, :, t, :]
                with tc.high_priority():
                    pp = ps_g.tile([P, 4 * B], fp32, tag="g1", name=f"pp{t}")
                    # r | z halves, bias row-matmul accumulated last
                    nc.tensor.matmul(pp[:, 0:B], lhsT=wih[:, 0:D], rhs=x_t,
                                     start=True, stop=False)
                    nc.tensor.matmul(pp[:, 0:B], lhsT=whh[:, 0:D], rhs=prev,
                                     start=False, stop=False)
                    nc.tensor.matmul(pp[:, B:2 * B], lhsT=wih[:, D:2 * D], rhs=x_t,
                                     start=True, stop=False)
                    nc.tensor.matmul(pp[:, B:2 * B], lhsT=whh[:, D:2 * D], rhs=prev,
                                     start=False, stop=False)
                    nc.tensor.matmul(pp[:, 0:2 * B], lhsT=brz2, rhs=sel2,
                                     start=False, stop=True)
                    # n: i_n | (h_n + b_hn)
                    nc.tensor.matmul(pp[:, 2 * B:3 * B], lhsT=wih[:, 2 * D:3 * D],
                                     rhs=x_t, start=True, stop=True)
                    nc.tensor.matmul(pp[:, 3 * B:4 * B], lhsT=whh[:, 2 * D:3 * D],
                                     rhs=prev, start=True, stop=False)
                    nc.tensor.matmul(pp[:, 3 * B:4 * B], lhsT=bhn, rhs=ones1[:],
                                     start=False, stop=True)
                    # r|z = sigmoid = 0.5*tanh(0.5*x) + 0.5 in one [P,2B] op
                    rz = hpool.tile([P, 2 * B], fp32, tag="rz")
                    nc.scalar.activation(out=rz[:], in_=pp[:, 0:2 * B],
                                         func=Act.Tanh, bias=zbias[:], scale=0.5)
                    nc.vector.tensor_scalar(out=rz[:], in0=rz[:], scalar1=0.5,
                                            scalar2=0.5, op0=Alu.mult, op1=Alu.add)
                    t1 = hpool.tile([P, B], fp32, tag="t1")
                    nc.vector.tensor_tensor(out=t1[:], in0=rz[:, 0:B],
                                            in1=pp[:, 3 * B:4 * B], op=Alu.mult)
                    t2 = hpool.tile([P, B], fp32, tag="t2")
                    nc.vector.tensor_tensor(out=t2[:], in0=t1[:],
                                            in1=pp[:, 2 * B:3 * B], op=Alu.add)
                    nt_ = hpool.tile([P, B], fp32, tag="nt")
                    nc.scalar.activation(out=nt_[:], in_=t2[:], func=Act.Tanh,
                                         bias=binv)
                    dd = hpool.tile([P, B], fp32, tag="dd")
                    nc.vector.tensor_tensor(out=dd[:], in0=prev, in1=nt_[:],
                                            op=Alu.subtract)
                    ee = hpool.tile([P, B], fp32, tag="ee")
                    nc.vector.tensor_tensor(out=ee[:], in0=rz[:, B:2 * B],
                                            in1=dd[:], op=Alu.mult)
                    hdst = userT[m][:, half * B:(half + 1) * B]
                    nc.vector.tensor_tensor(out=hdst, in0=nt_[:], in1=ee[:],
                                            op=Alu.add)
                    prev = hdst

                if half == 1:
                    ubf = hpool.tile([P, P], bf16, tag="ubf", name=f"ubf{m}")
                    nc.vector.tensor_copy(out=ubf[:], in_=userT[m][:])
                    exp_tiles[m] = emit_chunk(m, ubf)
                    gi_ = gi_of_chunk[m]
                    if m == groups[gi_][-1]:
                        emit_ar(gi_)
                        for mm_ in groups[gi_]:
                            emit_scale_out(mm_, exp_tiles.pop(mm_))

    nc.compile()
    return nc


def _prep_inputs(basket_items, basket_dollars, hidden, emb, W_ih, W_hh, b_ih, b_hh):
    emb = np.ascontiguousarray(np.asarray(emb, dtype=np.float32))
    items = np.asarray(basket_items).astype(np.int32)
    dollars = np.asarray(basket_dollars, dtype=np.float32)
    W_ihT = np.ascontiguousarray(np.asarray(W_ih, dtype=np.float32).T)  # [128, 384]
    W_hhT = np.ascontiguousarray(np.asarray(W_hh, dtype=np.float32).T)
    b_ih = np.asarray(b_ih, dtype=np.float32)
    b_hh = np.asarray(b_hh, dtype=np.float32)
    h0T = np.ascontiguousarray(np.asarray(hidden, dtype=np.float32)[0].T)  # [128, 64]

    # pooling pattern, periodic with lcm(128, 20) = 640 slots = 5 tiles:
    # tile t uses pat[:, t % 5, :] into psum window 32 * (t // 5).
    j = np.arange(5 * P)
    pat = np.zeros((P, 5, 32), dtype=np.float32)
    pat[j % P, j // P, j // K] = 1.0
    pat_t = pat[:, np.arange(NT) % 5, :]       # [P, NT, 32]

    in_maps = []
    for c in range(NC):
        items_c = items[c * BL:(c + 1) * BL]          # [8, S, K]
        dol_c = dollars[c * BL:(c + 1) * BL]
        idx_flat = items_c.transpose(1, 0, 2).reshape(-1)   # s-major slots
        dol_flat = dol_c.transpose(1, 0, 2).reshape(-1) * (1.0 / K)
        parity = (idx_flat & 1).astype(np.float32)
        pair_idx = (idx_flat >> 1).astype(np.int16)
        # dma_gather index layout: [16, n/16] with flat[c*16+p] at [p, c],
        # replicated across the 8 Q7 cores (rows 16..127).
        wrapped = pair_idx.reshape(SLOTS // 16, 16).T        # [16, n/16]
        gidx16 = np.ascontiguousarray(np.tile(wrapped, (8, 1)))  # [128, 200]
        gdole = (dol_flat * (1.0 - parity)).reshape(NT, P).T     # [P, NT]
        gdolo = (dol_flat * parity).reshape(NT, P).T

        mc = np.zeros((P, MC), dtype=np.float32)
        mc[:, PAT_E0:PAT_E0 + 800] = (pat_t * gdole[:, :, None]).reshape(P, 800)
        mc[:, PAT_O0:PAT_O0 + 800] = (pat_t * gdolo[:, :, None]).reshape(P, 800)
        mc[:, WIH0:WIH0 + 3 * D] = W_ihT
        mc[:, WHH0:WHH0 + 3 * D] = W_hhT
        mc[:, H00:H00 + B] = h0T
        mc[:, BIN0] = b_ih[2 * D:]
        mc[:, GIDX0:GIDX0 + 100] = gidx16.view(np.float32)
        mc[0, BRZ0:BRZ0 + P] = b_ih[0:D] + b_hh[0:D]
        mc[1, BRZ0:BRZ0 + P] = b_ih[D:2 * D] + b_hh[D:2 * D]
        mc[0, SEL0:SEL0 + B] = 1.0
        mc[1, SEL0 + B:SEL0 + 2 * B] = 1.0
        mc[0, BHN0:BHN0 + P] = b_hh[2 * D:]
        # per-core emb chunk in et layout (see CH0 comment)
        chunk = emb[c * VC:(c + 1) * VC]              # [6250, 128]
        mc[:, CH0:CH0 + 6144] = (
            chunk[:6144].reshape(6, 8, P, D).transpose(2, 0, 1, 3).reshape(P, 6144))
        mc[:VC - 6144, CT0:CT0 + D] = chunk[6144:]
        in_maps.append(dict(emb_full=emb, misc=mc))
    return in_maps


def kernel(basket_items, basket_dollars, hidden, emb, W_ih, W_hh, b_ih, b_hh,
           _want_trace=False):
    from concourse.bass_utils import run_bass_kernel_spmd

    if "nc" not in _CACHE:
        _CACHE["nc"] = _build()
    nc = _CACHE["nc"]

    in_maps = _prep_inputs(basket_items, basket_dollars, hidden, emb,
                           W_ih, W_hh, b_ih, b_hh)
    res = run_bass_kernel_spmd(nc, in_maps, core_ids=list(range(NC)),
                               trace=_want_trace)
    _CACHE["last_result"] = res
    out = np.concatenate([np.asarray(r["out_c"]) for r in res.results], axis=2)
    return np.ascontiguousarray(out.astype(np.float32))
